# revision 1
# baseline (speedup 1.0000x reference)
"""AdaptiveLocalPooling Trainium2 kernel (8 NeuronCores, batch-sharded).

For each (b, t): gather K=9 neighbor rows X[b, idx[t,k], :], cosine-sim
against X[b, t, :], softmax over K, weighted-pool the neighbors, then mean
over t -> cls [B, 1, C].

Per-core plan (B_local=2, T=4096, C=384, K=9):
  1. Pre-pass: build a bf16 "gather table" in DRAM: row j =
     [X[b0,j,:] bf16 | X[b1,j,:] bf16 | invnorm_b0 | invnorm_b1 | pad]
     (896 elems = 1792 B). Row norms are computed in f32 on DVE
     (tensor_tensor_reduce), inverted (DVE reciprocal + ACT sqrt), and also
     kept resident in SBUF for the query side.
  2. Main loop over 32 tiles of 128 t's:
       - gpsimd.dma_gather pulls the 9*128 neighbor rows (one 1792B
         descriptor covers both batches AND their inv-norms); 8-deep
         G/Q buffering keeps the SDMA gather stream running ahead of
         compute (the gather is the byte-bound bottleneck, ~145 GB/s
         effective on random 1792B reads).
       - queries come in via a plain sequential DMA of table rows.
       - dot[p,k,b] via fused scalar_tensor_tensor (mult+mult, accum_out),
         with the query inv-norm folded into the per-partition scalar.
       - sim2 = dots * gathered-ninv (strided TT); softmax: ACT Exp with
         fused per-batch accum_out denominators; DVE reciprocal;
         w = e * sinv (bf16).  All reduction-class outputs (STT/ACT
         accum_out, reciprocal) land LATE on HW and are only read >= ~19
         instructions after production (lag-3 phase pipeline).
       - pooled+mean: 18 PE matmuls [1x384] per tile accumulating
         sum_t sum_k w * X[idx] directly into PSUM across the whole
         kernel; epilogue scales by 1/T.
"""

import os
import sys

import numpy as np

for _p in ("/opt/trn_rl_repo", "/root/.axon_site/_ro/trn_rl_repo"):
    if os.path.isdir(_p) and _p not in sys.path:
        sys.path.insert(0, _p)

import concourse.bacc as bacc
import concourse.bass as bass
import concourse.mybir as mybir
from concourse.bass_utils import run_bass_kernel_spmd
from concourse.library_config import mlp

# Problem sizes (hardcoded per spec).
B = 16
T = 4096
C = 384
K = 9
N_CORES = 8
B_LOC = B // N_CORES  # 2

P = 128
NT = T // P  # 32 tiles
ROW = 2 * C + 2  # 770 payload elems per table row
ROW_PAD = 896  # padded to 1792 B (divisible by 256)
NIDX = K * P  # 1152 gathered rows per tile
IDX_COLS = NIDX // 16  # 72 idx columns per tile in the wrapped layout

FP32 = mybir.dt.float32
BF16 = mybir.dt.bfloat16
I16 = mybir.dt.int16
AX = mybir.AxisListType
OP = mybir.AluOpType
AF = mybir.ActivationFunctionType


def build_kernel(
    n_tiles: int = NT,
    debug: bool = False,
    pre_reps: int = 1,
    main_reps: int = 1,
    ablate: str = "",
) -> bass.Bass:
    # ablate: comma list of {stt, gather, pe} to stub out (timing-only builds)
    t_loc = n_tiles * P
    nc = bacc.Bacc("TRN2")

    x_ext = nc.declare_dram_parameter("X", [B_LOC, t_loc, C], FP32, isOutput=False)
    idx_ext = nc.declare_dram_parameter(
        "idx", [P, n_tiles * IDX_COLS], I16, isOutput=False
    )
    out_ext = nc.declare_dram_parameter("out", [B_LOC, C], FP32, isOutput=True)
    table = nc.dram_tensor("table", [t_loc, ROW_PAD], BF16)
    dbg = {}
    if debug:
        for nm, shape, dt in [
            ("d_ninv", [P, 2 * n_tiles], FP32),
            ("d_dots", [P, 2 * K], FP32),
            ("d_nvf", [P, 2 * K], FP32),
            ("d_sim2_0", [P, 2 * K], FP32),
            ("d_sim2_1", [P, 2 * K], FP32),
            ("d_e0", [P, 2 * K], FP32),
            ("d_e1", [P, 2 * K], FP32),
            ("d_s0", [P, 2], FP32),
            ("d_s1", [P, 2], FP32),
            ("d_w0", [P, 2 * K], BF16),
            ("d_w1", [P, 2 * K], BF16),
            ("d_q0", [P, ROW_PAD], BF16),
            ("d_q1", [P, ROW_PAD], BF16),
            ("d_g0", [P, K * ROW_PAD], BF16),
            ("d_g1", [P, K * ROW_PAD], BF16),
        ]:
            dbg[nm] = nc.declare_dram_parameter(nm, shape, dt, isOutput=True)

    from contextlib import ExitStack

    with ExitStack() as ctx:
        e = ctx.enter_context

        idx_sb = e(nc.sbuf_tensor("idx_sb", [P, n_tiles * IDX_COLS], I16))
        # pre-pass buffers (x2)
        xf = [e(nc.sbuf_tensor(f"xf{i}", [P, 2 * C], FP32)) for i in range(4)]
        row = [e(nc.sbuf_tensor(f"row{i}", [P, ROW_PAD], BF16)) for i in range(4)]
        nn = [e(nc.sbuf_tensor(f"nn{i}", [P, 2], FP32)) for i in range(4)]
        nrec = [e(nc.sbuf_tensor(f"nrec{i}", [P, 2], FP32)) for i in range(4)]
        ninv_sb = e(nc.sbuf_tensor("ninv_sb", [P, 2 * n_tiles], FP32))
        # main-loop buffers (G/Q x8 so gathers stream ahead; small ones x2)
        G = [e(nc.sbuf_tensor(f"G{i}", [P, K * ROW_PAD], BF16)) for i in range(8)]
        ghalf = (
            e(nc.sbuf_tensor("ghalf", [P, K * 512], BF16))
            if "halfbytes" in ablate
            else None
        )
        Q = [e(nc.sbuf_tensor(f"Q{i}", [P, ROW_PAD], BF16)) for i in range(8)]
        scr = e(nc.sbuf_tensor("scr", [P, C], BF16))
        dots = [e(nc.sbuf_tensor(f"dots{i}", [P, 2 * K], FP32)) for i in range(2)]
        sim2 = [e(nc.sbuf_tensor(f"sim2_{i}", [P, 2 * K], FP32)) for i in range(2)]
        ee = [e(nc.sbuf_tensor(f"e{i}", [P, 2 * K], FP32)) for i in range(2)]
        ss = [e(nc.sbuf_tensor(f"s{i}_", [P, 2], FP32)) for i in range(2)]
        sinv = e(nc.sbuf_tensor("sinv", [P, 2], FP32))
        dscr = e(nc.sbuf_tensor("dscr", [P, 2], FP32))
        ww = [e(nc.sbuf_tensor(f"w{i}", [P, 2 * K], BF16)) for i in range(4)]
        out_sb = e(nc.sbuf_tensor("out_sb", [33, C], FP32))
        acc = e(nc.psum_tensor("acc", [64, C], FP32))

        xload = [e(nc.semaphore(f"xload{i}")) for i in range(4)]
        vpre = e(nc.semaphore("vpre"))
        spre = e(nc.semaphore("spre"))
        tdone = [e(nc.semaphore(f"tdone{i}")) for i in range(4)]
        isem = e(nc.semaphore("isem"))
        gsem = [e(nc.semaphore(f"gsem{i}")) for i in range(8)]
        qsem = [e(nc.semaphore(f"qsem{i}")) for i in range(8)]
        v1 = e(nc.semaphore("v1"))
        v2 = e(nc.semaphore("v2"))
        aexp = e(nc.semaphore("aexp"))
        pe_done = e(nc.semaphore("pe_done"))
        vfin = e(nc.semaphore("vfin"))
        osem = e(nc.semaphore("osem"))

        block = e(nc.Block())
        n_pre = pre_reps * n_tiles
        n_main = main_reps * n_tiles

        def k3(ap):  # [P, 2K] -> [P, K, 2] (k-major pairs)
            return ap.rearrange("p (a b) -> p a b", b=2)

        def kT(ap):  # [P, 2K] -> [P, 2, 9] transposed view (reduce over k)
            return ap.rearrange("p (a b) -> p b a", b=2)

        def bcast2(ap2):  # [P, 2] -> [P, K, 2] with step-0 broadcast over K
            return ap2.rearrange("p (o b) -> p o b", o=1).to_broadcast([P, K, 2])

        @block.sync
        def _(sync: bass.BassEngine):
            # ---- pre-pass: X tile loads + table row stores ----
            for g in range(n_pre):
                t = g % n_tiles
                if g >= 4:
                    sync.wait_ge(vpre, g - 3)  # xf[g%4] free
                pp = g % 4
                sync.dma_start(
                    out=xf[pp][:].rearrange("p (b c) -> p b c", b=2),
                    in_=x_ext[:, t * P : (t + 1) * P, :].rearrange(
                        "b p c -> p b c"
                    ),
                ).then_inc(xload[pp], 16)
                if g >= 1:
                    tp = (g - 1) % n_tiles
                    sync.wait_ge(spre, g)  # row[g-1] fully assembled
                    sync.dma_start(
                        out=table[tp * P : (tp + 1) * P, :], in_=row[(g - 1) % 4][:]
                    ).then_inc(tdone[(g - 1) % 4], 16)
            sync.wait_ge(spre, n_pre)
            tp = (n_pre - 1) % n_tiles
            sync.dma_start(
                out=table[tp * P : (tp + 1) * P, :],
                in_=row[(n_pre - 1) % 4][:],
            ).then_inc(tdone[(n_pre - 1) % 4], 16)
            # ---- main loop: query loads (after full table resident) ----
            for j in range(4):
                cnt = (n_pre - 1 - j) // 4 + 1 if n_pre > j else 0
                if cnt:
                    sync.wait_ge(tdone[j], 16 * cnt)
            for g in range(n_main):
                t = g % n_tiles
                if g >= 8:
                    sync.wait_ge(v1, g - 6)  # Q[g%8] free (STTs g-7 done)
                sync.dma_start(
                    out=Q[g % 8][:, 0 : 2 * C],
                    in_=table[t * P : (t + 1) * P, 0 : 2 * C],
                ).then_inc(qsem[g % 8], 16)
            # ---- epilogue ----
            sync.wait_ge(vfin, 1)
            sync.dma_start(out=out_ext[0:1, :], in_=out_sb[0:1, :]).then_inc(osem, 16)
            sync.dma_start(out=out_ext[1:2, :], in_=out_sb[32:33, :]).then_inc(
                osem, 16
            )
            n_os = 32
            if debug:
                for nm, buf in [
                    ("d_ninv", ninv_sb), ("d_dots", dots[(n_tiles - 1) % 2]),
                    ("d_nvf", dots[0]),
                    ("d_sim2_0", sim2[0]), ("d_sim2_1", sim2[1]),
                    ("d_e0", ee[0]), ("d_e1", ee[1]),
                    ("d_s0", ss[0]), ("d_s1", ss[1]),
                    ("d_w0", ww[0]), ("d_w1", ww[1]),
                    ("d_q0", Q[0]), ("d_q1", Q[1]),
                    ("d_g0", G[0]), ("d_g1", G[1]),
                ]:
                    sync.dma_start(out=dbg[nm][:], in_=buf[:]).then_inc(osem, 16)
                    n_os += 16
            sync.wait_ge(osem, n_os)

        @block.vector
        def _(vector: bass.BassVectorEngine):
            # one-time: zero row pads so table pad bytes are defined
            for rbuf in row:
                vector.memset(rbuf[:, ROW:ROW_PAD], 0)
            # ---- pre-pass (reciprocal pipelined one tile behind: the fused
            # accum_out write lands late, so never read it immediately) ----
            for g in range(n_pre):
                pp = g % 4
                vector.wait_ge(xload[pp], 16 * (g // 4 + 1))
                if g >= 4:
                    vector.wait_ge(tdone[pp], 16 * (g // 4))  # row[g%4] stored
                for b in range(2):
                    vector.tensor_copy(
                        out=row[pp][:, b * C : (b + 1) * C],
                        in_=xf[pp][:, b * C : (b + 1) * C],
                    )
                    vector.scalar_tensor_tensor(
                        out=scr[:],
                        in0=xf[pp][:, b * C : (b + 1) * C],
                        scalar=1.0,
                        in1=xf[pp][:, b * C : (b + 1) * C],
                        op0=OP.mult,
                        op1=OP.mult,
                        accum_out=nn[pp][:, b : b + 1],
                    )
                if g >= 1:
                    qq = (g - 1) % 4
                    vector.reciprocal(out=nrec[qq][:], in_=nn[qq][:]).then_inc(
                        vpre, 1
                    )
            # spacers to let the last nn accum land before the tail recip
            vector.tensor_copy(out=scr[:], in_=row[(n_pre - 1) % 4][:, 0:C])
            vector.tensor_copy(out=scr[:], in_=row[(n_pre - 1) % 4][:, 0:C])
            qq = (n_pre - 1) % 4
            vector.reciprocal(out=nrec[qq][:], in_=nn[qq][:]).then_inc(vpre, 1)

            # ---- main loop ----
            # iteration t: [A1] recip for t-2, [C] dots for t, [B] sim2 for
            # t-1, [A2] weights for t-2.  Reduction-class outputs (STT
            # accum_out, ACT accum_out, reciprocal) land late on HW, so every
            # such value is read >= ~19 instructions after it is produced.
            def phase_a1(m):  # sinv = 1/ss for tile m (ss from ACT accum)
                vector.wait_ge(aexp, m + 1)
                vector.reciprocal(out=sinv[:], in_=ss[m % 2][:])

            def phase_b(m):  # sim2 for tile m (reads dots[m%2], late-landing)
                gv = G[m % 8][:].rearrange("p (g r) -> p g r", r=ROW_PAD)
                vector.tensor_tensor(
                    out=k3(sim2[m % 2][:]),
                    in0=k3(dots[m % 2][:]),
                    in1=gv[:, :, 2 * C : 2 * C + 2],
                    op=OP.mult,
                ).then_inc(v1, 1)

            def phase_a2(m):  # ww = ee * sinv for tile m
                if m >= 4:
                    vector.wait_ge(pe_done, m - 3)  # ww[m%4] free
                vector.tensor_tensor(
                    out=k3(ww[m % 4][:]),
                    in0=k3(ee[m % 2][:]),
                    in1=bcast2(sinv[:]),
                    op=OP.mult,
                ).then_inc(v2, 1)

            def spacer(n=2):
                for _ in range(n):
                    vector.tensor_copy(out=scr[:], in_=row[0][:, 0:C])

            for g in range(n_main):
                t = g % n_tiles
                if g >= 3:
                    phase_a1(g - 3)
                vector.wait_ge(gsem[g % 8], 32 * (g // 8 + 1))
                vector.wait_ge(qsem[g % 8], 16 * (g // 8 + 1))
                for k in range(1 if "stt" in ablate else K):
                    for b in range(2):
                        vector.scalar_tensor_tensor(
                            out=scr[:],
                            in0=Q[g % 8][:, b * C : (b + 1) * C],
                            scalar=ninv_sb[:, 2 * t + b : 2 * t + b + 1],
                            in1=G[g % 8][
                                :, k * ROW_PAD + b * C : k * ROW_PAD + (b + 1) * C
                            ],
                            op0=OP.mult,
                            op1=OP.mult,
                            accum_out=dots[g % 2][:, k * 2 + b : k * 2 + b + 1],
                        )
                if g >= 1:
                    phase_b(g - 1)
                if g >= 3:
                    phase_a2(g - 3)
            # tail (in-loop phases covered m <= n_main-4)
            for m in (n_main - 3, n_main - 2, n_main - 1):
                if m < 0 or m <= n_main - 4:
                    continue
                if m == n_main - 1:
                    spacer()
                    phase_b(m)
                phase_a1(m)
                spacer(3)
                phase_a2(m)
            # epilogue: PSUM -> SBUF with 1/T scaling
            vector.wait_ge(pe_done, n_main)
            vector.tensor_scalar_mul(
                out=out_sb[0:1, :], in0=acc[0:1, :], scalar1=1.0 / (t_loc * main_reps)
            )
            vector.tensor_scalar_mul(
                out=out_sb[32:33, :], in0=acc[32:33, :], scalar1=1.0 / (t_loc * main_reps)
            ).then_inc(vfin, 1)

        @block.scalar
        def _(scalar: bass.BassScalarEngine):
            # ---- pre-pass: ninv = sqrt(1/nn), to resident f32 + bf16 row tail
            for g in range(n_pre):
                t = g % n_tiles
                pp = g % 4
                scalar.wait_ge(vpre, g + 1)
                scalar.activation(
                    out=ninv_sb[:, 2 * t : 2 * t + 2], in_=nrec[pp][:], func=AF.Sqrt
                )
                scalar.activation(
                    out=row[pp][:, 2 * C : 2 * C + 2],
                    in_=ninv_sb[:, 2 * t : 2 * t + 2],
                    func=AF.Copy,
                ).then_inc(spre, 1)
            # ---- main loop: exp with fused per-batch denominators; the
            # aexp inc rides a trailing dummy op so the accum lands first ----
            for g in range(n_main):
                scalar.wait_ge(v1, g + 1)
                if g >= 2:
                    scalar.wait_ge(v2, g - 1)  # ee/ss[g%2] free (a2 lag 3)
                s23 = k3(sim2[g % 2][:])
                e3 = k3(ee[g % 2][:])
                for b in range(2):
                    scalar.activation(
                        out=e3[:, :, b : b + 1],
                        in_=s23[:, :, b : b + 1],
                        func=AF.Exp,
                        accum_out=ss[g % 2][:, b : b + 1],
                    )
                scalar.activation(
                    out=dscr[:], in_=sim2[g % 2][:, 0:2], func=AF.Copy
                ).then_inc(aexp, 1)

        @block.tensor
        def _(tensor: bass.BassTensorEngine):
            for g in range(n_main):
                tensor.wait_ge(v2, g + 1)
                for k in range(1 if "pe" in ablate else K):
                    for b in range(2):
                        mm = tensor.matmul(
                            out=acc[32 * b : 32 * b + 1, :],
                            lhsT=ww[g % 4][:, k * 2 + b : k * 2 + b + 1],
                            rhs=G[g % 8][
                                :, k * ROW_PAD + b * C : k * ROW_PAD + (b + 1) * C
                            ],
                            start=(g == 0 and k == 0),
                            stop=(g == n_main - 1 and k == K - 1),
                            skip_group_check=True,
                        )
                mm.then_inc(pe_done, 1)

        @block.gpsimd
        def _(gpsimd: bass.BassGpSimd):
            gpsimd.load_library(mlp)
            gpsimd.dma_start(out=idx_sb[:], in_=idx_ext[:]).then_inc(isem, 16)
            gpsimd.wait_ge(isem, 16)
            for j in range(4):
                cnt = (n_pre - 1 - j) // 4 + 1 if n_pre > j else 0
                if cnt:
                    gpsimd.wait_ge(tdone[j], 16 * cnt)  # full table resident
            for g in range(n_main):
                t = g % n_tiles
                if g >= 8:
                    gpsimd.wait_ge(pe_done, g - 7)  # G[g%8] free
                gv3 = G[g % 8][:].rearrange("p (g r) -> p g r", r=ROW_PAD)
                # 1024+128 split: the large first call keeps the SDMA
                # stream busier across call boundaries (~20% faster than 5+4)
                spans = ((0, 1), (1, 2)) if "gather" in ablate else ((0, 8), (8, K))
                sp_flag = True
                for g0, g1 in spans:
                    n = (g1 - g0) * P
                    if "halfbytes" in ablate:
                        gpsimd.dma_gather(
                            ghalf[:].rearrange("p (g r) -> p g r", r=512)[
                                :, g0:g1, :
                            ],
                            table[:, 0:512],
                            idx_sb[
                                :,
                                t * IDX_COLS + g0 * 8 : t * IDX_COLS + g1 * 8,
                            ],
                            n,
                            n,
                            512,
                            elem_step=ROW_PAD,
                        ).then_inc(gsem[g % 8], 16)
                    else:
                        gpsimd.dma_gather(
                            gv3[:, g0:g1, :],
                            table[:],
                            idx_sb[
                                :,
                                t * IDX_COLS + g0 * 8 : t * IDX_COLS + g1 * 8,
                            ],
                            n,
                            n,
                            ROW_PAD,
                            single_packet=sp_flag,
                        ).then_inc(gsem[g % 8], 16)

    nc.compile()
    return nc


def make_idx_table(neighbor_idx: np.ndarray, n_tiles: int = NT) -> np.ndarray:
    """Host-side index preprocessing into dma_gather's wrapped int16 layout.

    Flat order per tile: i = k*128 + p  ->  neighbor_idx[t0+p, k].
    Wrapped: idx_sb[q, tile*IDX_COLS + c] = flat[c*16 + q%16].
    """
    nb = np.asarray(neighbor_idx).astype(np.int16)  # values < 4096
    cols = np.empty((P, n_tiles * IDX_COLS), dtype=np.int16)
    for t in range(n_tiles):
        flat = nb[t * P : (t + 1) * P, :].T.reshape(-1)  # [K*P], k-major
        wrap = flat.reshape(IDX_COLS, 16).T  # [16, IDX_COLS]
        cols[:, t * IDX_COLS : (t + 1) * IDX_COLS] = np.tile(wrap, (8, 1))
    return cols


_NC_CACHE: dict = {}


def _get_nc():
    if "nc" not in _NC_CACHE:
        _NC_CACHE["nc"] = build_kernel()
    return _NC_CACHE["nc"]


def kernel(X: np.ndarray, neighbor_idx: np.ndarray, **_ignored) -> np.ndarray:
    X = np.asarray(X, dtype=np.float32)
    idx_cols = make_idx_table(neighbor_idx)
    nc = _get_nc()
    core_ids = list(range(N_CORES))
    in_maps = [
        {"X": np.ascontiguousarray(X[i * B_LOC : (i + 1) * B_LOC]), "idx": idx_cols}
        for i in core_ids
    ]
    res = run_bass_kernel_spmd(nc, in_maps, core_ids)
    outs = [res.results[i]["out"] for i in range(N_CORES)]  # each [B_LOC, C]
    full = np.concatenate(outs, axis=0).reshape(B, 1, C).astype(np.float32)
    return full


if __name__ == "__main__":
    rng = np.random.default_rng(0)
    X = rng.standard_normal((B, T, C), dtype=np.float32)
    nb = rng.integers(0, T, size=(T, K)).astype(np.int64)
    out = kernel(X, nb)
    print("out", out.shape, out.dtype, float(np.abs(out).mean()))

